# revision 1
# baseline (speedup 1.0000x reference)
"""Causal multi-head attention (B=2, S=2048, D=1024, H=16, hd=64) on 8 trn2 cores.

Sharding: core = (batch b, head-group g): cores 0-3 -> batch 0, groups 0-3;
cores 4-7 -> batch 1. Each core computes 4 heads of one batch element:
QKV projections for its 256 hd-dims, causal attention, and a partial output
projection (attn_heads @ Wo.T restricted to its hd columns). Host sums the 4
partials per batch (bf16 partials) and adds bo.

Fully software-pipelined over 512-token chunks t: proj(t+1) cell/transpose
units are interleaved into attention(t)'s k-block loop so the PE fills
exp-wait stalls with projection matmuls; outproj(t-1) runs between the two
head-pairs of attention(t). All PE matmuls bf16. Causal mask = restricted
diagonal score matmuls + [128,2,128] 0/1 triangle multiply on probs (DVE);
dead prob columns memset so PV runs full width. Scores for both heads of a
pair land in one [128,2,512] PSUM tile -> ONE exp per k-block. PSUM:
"gen" ring (4x1 bank: proj cells, V transposes, outproj, PV accumulators)
+ "sc" ring (2x2 banks). Normalize: attn rows copied out to SBUF early
(frees PV psum fast), denom row -> DRAM bounce -> packed [128,8]
reciprocal (bf16) -> partition-broadcast read; the final multiplies are
DEFERRED one attention-pair (drain-keep-1 FIFO) so the in-order DVE queue
never stalls waiting on the DMA bounce. Queue split: x-input/output/
broadcast/shift DMAs issue from the GpSimd queue, the bounce from Sync,
so no queue couples latency into another engine's work.
Measured: ~202 us HW exec (8 cores), rel err ~4e-3 (baseline was 220 us).
"""
import sys

sys.path.insert(0, "/opt/trn_rl_repo")

import numpy as np
import ml_dtypes

import concourse.bass as bass
import concourse.bacc as bacc
import concourse.tile as tile
import concourse.mybir as mybir
from concourse.bass_utils import run_bass_kernel_spmd

B, S, D, H, HD = 2, 2048, 1024, 16, 64
HPC = 4            # heads per core
HDC = HPC * HD     # 256 hd dims per core
KC = D // 128      # 8 contraction chunks
TQ = S // 512      # 4 q-chunks of 512
SCALE = 1.0 / 8.0  # 1/sqrt(64)

f32 = mybir.dt.float32
f32r = mybir.dt.float32r
bf16 = mybir.dt.bfloat16

_CACHE = {}


def _emit(tc, d, ctx):
    nc = tc.nc
    singles = ctx.enter_context(tc.tile_pool(name="singles", bufs=1))
    xt_pool = ctx.enter_context(tc.tile_pool(name="xt", bufs=2))
    qt_pool = ctx.enter_context(tc.tile_pool(name="qt", bufs=2))
    vtt_pool = ctx.enter_context(tc.tile_pool(name="vtt", bufs=2))
    pr_pool = ctx.enter_context(tc.tile_pool(name="pr", bufs=3))
    norm_pool = ctx.enter_context(tc.tile_pool(name="norm", bufs=2))
    stage_pool = ctx.enter_context(tc.tile_pool(name="stage", bufs=2))
    gen = ctx.enter_context(tc.tile_pool(name="gen", bufs=4, space="PSUM"))
    psc = ctx.enter_context(tc.tile_pool(name="psc", bufs=2, space="PSUM"))

    xd = {
        nm: d[nm][:].rearrange("(c p) s -> p c s", c=KC) for nm in ("xq", "xk", "xv")
    }

    # --- preamble DMAs: weights/consts on Sync queue, x(0) on GpSimd queue
    w_sb = {}

    def wload(wnm):
        w_sb[wnm] = singles.tile([128, KC, HDC], bf16, tag=wnm, name=wnm)
        nc.sync.dma_start(
            out=w_sb[wnm], in_=d[wnm][:].rearrange("p (kc m) -> p kc m", kc=KC)
        )

    state = {}

    def prep(t):
        """Allocate x/qt/vtt tiles for chunk t and issue x DMAs (GpSimd q)."""
        tsl = slice(t * 512, (t + 1) * 512)
        xts = {}
        for nm in ("xk", "xq", "xv"):
            xts[nm] = xt_pool.tile([128, KC, 512], bf16, tag=nm, name=f"{nm}_{t}")
            if t == 0:  # chunk-split so the first proj can start early
                for c in range(KC):
                    nc.gpsimd.dma_start(out=xts[nm][:, c, :], in_=xd[nm][:, c, tsl])
            else:
                nc.gpsimd.dma_start(out=xts[nm], in_=xd[nm][:, :, tsl])
        qt = qt_pool.tile([128, 2, 512], bf16, tag="qt", name=f"qt_{t}")
        vtt = vtt_pool.tile([128, 2, 512], f32r, tag="vtt", name=f"vtt_{t}")
        state[t] = (xts, qt, vtt)

    wload("wk")
    wload("wq")
    prep(0)
    bias_sb = singles.tile([128, 6], f32)
    nc.sync.dma_start(out=bias_sb, in_=d["bias"][:])
    wload("wv")
    trimask = singles.tile([128, 2, 128], bf16)
    nc.sync.dma_start(
        out=trimask, in_=d["trimask"][:].rearrange("p (a b) -> p a b", a=2)
    )
    ident = singles.tile([128, 128], f32r)
    nc.sync.dma_start(out=ident, in_=d["ident"][:])
    wo_sb = singles.tile([128, 2, D], bf16)
    nc.sync.dma_start(out=wo_sb, in_=d["wo"][:].rearrange("p (c o) -> p c o", c=2))

    # persistent attention state
    ktz_sb = singles.tile([128, 2, 2, S], bf16, tag="ktz")
    nc.vector.memset(ktz_sb[64:128, :, 0, :], 0.0)
    nc.vector.memset(ktz_sb[0:64, :, 1, :], 0.0)
    v_sb = [
        singles.tile([128, S // 128, 65], bf16, tag=f"v{h}", name=f"v{h}")
        for h in range(HPC)
    ]
    for h in range(HPC):
        nc.vector.memset(v_sb[h][:, :, 64:65], 1.0)
    attnt_sb = singles.tile([128, 2, S], bf16, tag="attnt")

    def proj_units(t):
        """Yield proj work units for chunk t: 6 cells + 8 transposes."""
        xts, qt, vtt = state[t]
        tsl = slice(t * 512, (t + 1) * 512)

        def k_dst(mc, cell):
            nc.scalar.add(
                out=ktz_sb[0:64, mc, 0, tsl],
                in_=cell[0:64, :],
                add=bias_sb[0:64, 2 + mc : 3 + mc],
            )
            nc.scalar.add(
                out=ktz_sb[64:128, mc, 1, tsl],
                in_=cell[64:128, :],
                add=bias_sb[64:128, 2 + mc : 3 + mc],
            )

        def q_dst(mc, cell):
            nc.scalar.add(out=qt[:, mc, :], in_=cell, add=bias_sb[:, mc : mc + 1])

        def v_dst(mc, cell):
            nc.vector.tensor_scalar_add(
                out=vtt[:, mc, :], in0=cell, scalar1=bias_sb[:, 4 + mc : 5 + mc]
            )

        def cell_unit(xnm, wnm, mc, dst_fn):
            def run():
                cell = gen.tile([128, 512], f32, tag="gen", name=f"cell_{wnm}_{t}_{mc}")
                for c in range(KC):
                    nc.tensor.matmul(
                        cell,
                        w_sb[wnm][:, c, mc * 128 : (mc + 1) * 128],
                        xts[xnm][:, c, :],
                        start=(c == 0),
                        stop=(c == KC - 1),
                    )
                dst_fn(mc, cell)

            return run

        def tp_unit(mc, tb):
            def run():
                tp = gen.tile([128, 512], f32r, tag="gen", name=f"tp_{t}_{mc}_{tb}")
                with nc.allow_low_precision(reason="f32r transpose; psum is fp32"):
                    nc.tensor.transpose(
                        tp[:, 0:128], vtt[:, mc, tb * 128 : (tb + 1) * 128], ident
                    )
                for h2 in range(2):
                    h = 2 * mc + h2
                    with nc.allow_low_precision(reason="V in bf16 for PV matmul"):
                        nc.vector.tensor_copy(
                            out=v_sb[h][:, 4 * t + tb, 0:64],
                            in_=tp[:, h2 * 64 : (h2 + 1) * 64],
                        )

            return run

        for xnm, wnm, dst in (
            ("xk", "wk", k_dst),
            ("xq", "wq", q_dst),
            ("xv", "wv", v_dst),
        ):
            for mc in range(2):
                yield cell_unit(xnm, wnm, mc, dst)
        for mc in range(2):
            for tb in range(4):
                yield tp_unit(mc, tb)

    def outproj(t):
        for tb in range(4):
            i = 4 * t + tb
            ot = stage_pool.tile([128, 2, 512], bf16, tag="ot")
            for o in range(2):
                po = gen.tile([128, 512], f32, tag="gen", name=f"po_{i}_{o}")
                for c in range(2):
                    nc.tensor.matmul(
                        po,
                        attnt_sb[:, c, i * 128 : (i + 1) * 128],
                        wo_sb[:, c, o * 512 : (o + 1) * 512],
                        start=(c == 0),
                        stop=(c == 1),
                    )
                nc.vector.tensor_copy(out=ot[:, o, :], in_=po)
            nc.gpsimd.dma_start(
                out=d["out"][i * 128 : (i + 1) * 128, :],
                in_=ot[:].rearrange("p a b -> p (a b)"),
            )

    def attention(t, units, finishes, mid_hooks):
        _, qt, _ = state[t]
        tsl = slice(t * 512, (t + 1) * 512)
        nkb = 4 * t + 4
        for p in range(2):
            if p == 1:
                for hook in mid_hooks:
                    hook()
            pvt = [
                gen.tile([128, 512], f32, tag="gen", name=f"pv_{t}_{p}_{h2}")
                for h2 in range(2)
            ]
            pending = None  # software pipeline: PV one kb behind scores
            for kb in range(nkb):
                dg = kb - 4 * t
                lo = 128 * dg if dg > 0 else 0
                scg = psc.tile([128, 2, 512], f32, tag="sc", name=f"sc_{t}_{p}_{kb}")
                for h2 in range(2):
                    nc.tensor.matmul(
                        scg[:, h2, lo:512],
                        ktz_sb[:, p, h2, kb * 128 : (kb + 1) * 128],
                        qt[:, p, lo:512],
                        start=True,
                        stop=True,
                    )
                pr = pr_pool.tile([128, 2, 512], bf16, tag="pr", name=f"pr_{t}_{p}_{kb}")
                if lo > 0:
                    nc.vector.memset(pr[:, :, 0:lo], 0.0)
                nc.scalar.activation(
                    out=pr[:, :, lo:512],
                    in_=scg[:, :, lo:512],
                    func=mybir.ActivationFunctionType.Exp,
                    scale=SCALE,
                )
                if dg >= 0:
                    nc.vector.tensor_tensor(
                        out=pr[:, :, lo : lo + 128],
                        in0=pr[:, :, lo : lo + 128],
                        in1=trimask[:],
                        op=mybir.AluOpType.mult,
                    )
                if pending is not None:
                    pkb, ppr = pending
                    for h2 in range(2):
                        nc.tensor.matmul(
                            pvt[h2][0:65, :],
                            v_sb[2 * p + h2][:, pkb, :],
                            ppr[:, h2, :],
                            start=(pkb == 0),
                            stop=False,
                        )
                pending = (kb, pr)
                if units:
                    units.pop(0)()
            pkb, ppr = pending
            for h2 in range(2):
                nc.tensor.matmul(
                    pvt[h2][0:65, :],
                    v_sb[2 * p + h2][:, pkb, :],
                    ppr[:, h2, :],
                    start=(pkb == 0),
                    stop=True,
                )

            # ---- normalize; copy attn rows out early to free PV psum ----
            av = [
                norm_pool.tile([64, 512], bf16, tag=f"av{h2}", name=f"av_{p}_{h2}")
                for h2 in range(2)
            ]
            dn = norm_pool.tile([65, 2, 512], f32, tag="dn")
            for h2 in range(2):
                nc.vector.tensor_copy(out=dn[64:65, h2, :], in_=pvt[h2][64:65, :])
                with nc.allow_low_precision(reason="attn in bf16"):
                    nc.vector.tensor_copy(out=av[h2][:], in_=pvt[h2][0:64, :])
            nc.sync.dma_start(out=d["nscr"][p, t], in_=dn[64:65, :, :])
            wide = norm_pool.tile([128, 8], f32, tag="wide")
            flat_in = d["nscr"][p, t].rearrange("c q -> (c q)").rearrange(
                "(pp f) -> pp f", pp=128
            )
            nc.sync.dma_start(out=wide[:], in_=flat_in)
            wrec = norm_pool.tile([128, 8], bf16, tag="wrec")
            with nc.allow_low_precision(reason="softmax denominators; bf16 recip"):
                nc.vector.reciprocal(out=wrec[:], in_=wide[:])
            flat_out = d["nscr2"][p, t].rearrange("c q -> (c q)").rearrange(
                "(pp f) -> pp f", pp=128
            )
            nc.sync.dma_start(out=flat_out, in_=wrec[:])
            bc = norm_pool.tile([64, 2, 512], bf16, tag="bc")
            for h2 in range(2):
                srcd = d["nscr2"][p, t, h2, :]
                rep = bass.AP(
                    tensor=srcd.tensor,
                    offset=srcd.offset,
                    ap=[[0, 64]] + [list(e) for e in srcd.ap],
                )
                nc.gpsimd.dma_start(out=bc[:, h2, :], in_=rep)

            # The final multiplies wait on the DMA bounce; defer them (via the
            # returned closure) so the in-order DVE queue never stalls on DMA.
            # The head-B partition-shift DMA issues from the GpSimd queue so it
            # is not stuck behind the next pair's bounce on the Sync queue.
            def finish(p=p, av=av, bc=bc):
                tmpb = norm_pool.tile([64, 512], bf16, tag="tmpb")
                with nc.allow_low_precision(reason="attn in bf16"):
                    nc.vector.tensor_tensor(
                        out=attnt_sb[0:64, p, tsl],
                        in0=av[0][:],
                        in1=bc[:, 0, :],
                        op=mybir.AluOpType.mult,
                    )
                    nc.vector.tensor_tensor(
                        out=tmpb[:],
                        in0=av[1][:],
                        in1=bc[:, 1, :],
                        op=mybir.AluOpType.mult,
                    )
                nc.gpsimd.dma_start(out=attnt_sb[64:128, p, tsl], in_=tmpb[:])
                nc.vector.tensor_scalar_add(
                    out=attnt_sb[:, p, tsl],
                    in0=attnt_sb[:, p, tsl],
                    scalar1=bias_sb[:, 4 + p : 5 + p],
                )

            finishes.append(finish)
        # flush any proj units not consumed by the kb loops
        while units:
            units.pop(0)()

    # t=0: proj fully up front
    for u in proj_units(0):
        u()
    finishes = []

    def drain(keep):
        # run deferred normalizes, always leaving `keep` pending so every
        # finish executes well after its DMA bounce was issued
        while len(finishes) > keep:
            finishes.pop(0)()

    for t in range(TQ):
        if t + 1 < TQ:
            prep(t + 1)
            units = list(proj_units(t + 1))
        else:
            units = []
        drain(1)
        mid = [lambda: drain(1)]
        if t > 0:
            mid.append(lambda t=t: outproj(t - 1))
        attention(t, units, finishes, mid)
    drain(0)
    outproj(TQ - 1)


def _build_nc():
    nc = bacc.Bacc()
    d = {}
    for nm in ("xq", "xk", "xv"):
        d[nm] = nc.declare_dram_parameter(nm, [D, S], bf16, isOutput=False)
    for nm in ("wq", "wk", "wv"):
        d[nm] = nc.declare_dram_parameter(nm, [128, KC * HDC], bf16, isOutput=False)
    d["wo"] = nc.declare_dram_parameter("wo", [128, 2 * D], bf16, isOutput=False)
    d["bias"] = nc.declare_dram_parameter("bias", [128, 6], f32, isOutput=False)
    d["trimask"] = nc.declare_dram_parameter("trimask", [128, 2 * 128], bf16, isOutput=False)
    d["ident"] = nc.declare_dram_parameter("ident", [128, 128], f32r, isOutput=False)
    d["out"] = nc.declare_dram_parameter("out", [S, D], bf16, isOutput=True)
    d["nscr"] = nc.dram_tensor("nscr", [2, TQ, 2, 512], f32)
    d["nscr2"] = nc.dram_tensor("nscr2", [2, TQ, 2, 512], bf16)
    from contextlib import ExitStack

    with tile.TileContext(nc) as tc:
        with ExitStack() as ctx:
            _emit(tc, d, ctx)
    nc.compile()
    return nc


def _get_nc():
    if "nc" not in _CACHE:
        _CACHE["nc"] = _build_nc()
    return _CACHE["nc"]


def _xarr(xt):
    return np.ascontiguousarray(xt).astype(ml_dtypes.bfloat16)


def _warr(wt):  # [D, HDC] -> [128, KC*HDC] chunk-contiguous
    return np.ascontiguousarray(
        wt.reshape(KC, 128, HDC).transpose(1, 0, 2).reshape(128, KC * HDC)
    ).astype(ml_dtypes.bfloat16)


def _woarr(wt):  # [HDC, D] -> [128, 2*D]
    return np.ascontiguousarray(
        wt.reshape(2, 128, D).transpose(1, 0, 2).reshape(128, 2 * D)
    ).astype(ml_dtypes.bfloat16)


def _host_consts():
    p = np.arange(128)[:, None]
    j = np.arange(128)[None, :]
    tri = (p <= j).astype(ml_dtypes.bfloat16)
    trimask = np.concatenate([tri, tri], axis=1)  # [128, 2*128], h2-duplicated
    ident = np.eye(128, dtype=np.float32)
    return trimask, ident


def kernel(trace=False, **inputs):
    q = np.asarray(inputs["q"], np.float32)
    k = np.asarray(inputs["k"], np.float32)
    v = np.asarray(inputs["v"], np.float32)
    Wq = np.asarray(inputs["Wq"], np.float32)
    Wk = np.asarray(inputs["Wk"], np.float32)
    Wv = np.asarray(inputs["Wv"], np.float32)
    Wo = np.asarray(inputs["Wo"], np.float32)
    bq = np.asarray(inputs["bq"], np.float32)
    bk = np.asarray(inputs["bk"], np.float32)
    bv = np.asarray(inputs["bv"], np.float32)
    bo = np.asarray(inputs["bo"], np.float32)
    # inputs["mask"] is the causal tril mask, baked into the kernel.

    trimask, ident = _host_consts()
    nc = _get_nc()
    in_maps = []
    for core in range(8):
        b, g = core // 4, core % 4
        sl = slice(g * HDC, (g + 1) * HDC)
        bias = np.zeros((128, 6), np.float32)
        for col, bvec in ((0, bq), (2, bk), (4, bv)):
            seg = bvec[sl].reshape(2, 128)
            bias[:, col] = seg[0]
            bias[:, col + 1] = seg[1]
        in_maps.append(
            {
                "xq": _xarr(q[b].T),
                "xk": _xarr(k[b].T),
                "xv": _xarr(v[b].T),
                "wq": _warr(Wq[sl, :].T),
                "wk": _warr(Wk[sl, :].T),
                "wv": _warr(Wv[sl, :].T),
                "wo": _woarr(Wo[:, sl].T),
                "bias": bias,
                "trimask": trimask,
                "ident": ident,
            }
        )
    res = run_bass_kernel_spmd(nc, in_maps, core_ids=list(range(8)), trace=trace)
    outs = [np.asarray(r["out"], np.float32) for r in res.results]
    final = np.empty((B, S, D), np.float32)
    for b in range(B):
        final[b] = outs[4 * b] + outs[4 * b + 1] + outs[4 * b + 2] + outs[4 * b + 3]
        final[b] += bo
    if trace:
        kernel.last_exec_time_ns = res.exec_time_ns
        kernel.last_results = res
    return final

